# revision 1
# baseline (speedup 1.0000x reference)
"""BERT self-attention (B=8, S=1024, D=768, H=12) on 8 TRN2 NeuronCores.

Strategy
--------
Data-parallel over batch: core b handles batch element b (no collectives).

Per core (layouts keep the contraction dim in the partition axis):

  1. mixT[e, s] = sum_d W^T[d, e] * x^T[d, s] + bias[e]: bf16 matmuls,
     fp32 psum, bias added during the DVE evacuation to bf16 mixbf.
  2. Q=K=V => scores are symmetric; the exp'd tile in [t, s] layout equals
     the transposed (unnormalized) probability matrix. scores chunk =
     Z_h^T @ MIX (Z zero-masks the other head of the e-pair), fp32 psum.
     Band-limited symmetry (R=2): chunk i only computes/exps columns
     >= 128*(i-R); the skipped tiles are EXACT PE-transposes of their
     symmetric counterparts (both orientations produce bitwise-identical
     scores), cutting ACT work by 21%.
  3. exp: 96 ACTIVATEs (banded widths) -> u tiles bf16 in SBUF; no bias
     needed (see mask trick below).
  4. ctx with U as the STATIONARY operand: out[s, dh] = sum_t U[t,s]*xl[t,dh]
     via matmul(lhsT=U(i, sc-chunk)[128,128], rhs=xl_i[128, 65]) accumulating
     8 s-chunk psum slots [128, 65] (ones column -> denominator col 64).
     128-col bf16 stationaries get Fast-Weight-Load (~64cyc, hidden under
     the 65-col matmul), so ctx costs ~65 cyc/tile vs 128 for the moving-U
     form — and the output is already in [s, dh] layout: NO ctx transposes,
     NO [65,512] psum evacuation casts.
  5. mask (additive, per-key t) is folded EXACTLY into xl: xl rows (values
     and the ones column) are scaled by emask[t] = exp(mask[t]) when xl is
     built from the mixT transposes (tensor_scalar_mul instead of copy).
     exp(s/8 + m[t]) * v[t] == exp(s/8) * (e^{m[t]} v[t]) and likewise for
     the denominator, so softmax stays exact for any finite/-inf mask.
  6. Epilogue per head: one strided [128, 2, 4] denominator extract from
     the ctx psum slots' col 64, one reciprocal, 8 tensor_scalar_mul
     [128, 64] normalizations straight into the staging tile.
  7. Output flushed as [128, k*64] DMAs (1-1.5KB contiguous runs) in
     groups (heads 0-6, 6-11, 11-12) so the tail flush is tiny.

Scheduling: ACT (softmax exp, ~94us banded) is the steady-state pacer. Emission
order: scores+exp of head h before ctx of head h-1 (deferred-ctx), the
projection/prep of pair j+1 between them, and the final head's ctx
+ epilogue trailing the last ACTIVATE.

Measured on TRN2 (8 cores): ~132.8 us HW exec (baseline 140.5), rel err
~5.7e-3 vs the fp32 reference.
"""

import numpy as np

import concourse.bacc as bacc
import concourse.tile as tile
from concourse import mybir
from concourse.bass_utils import run_bass_kernel_spmd

B, S, D = 8, 1024, 768
H, DH = 12, 64
NP = 6            # e-tile pairs (2 heads each)
NT = 8            # t-chunks / s-chunks of 128
R = 2             # symmetry band: exp computes cols >= 128*(i-R) of chunk i;
                  # farther tiles come from PE transposes of the symmetric
                  # counterpart (exact: both orientations compute bitwise-
                  # identical scores).
C0 = [128 * max(0, i - R) for i in range(NT)]
# transposed-tile index: source chunk s provides U^T tiles for dest chunks
# i > s + R; ctx slot sc with source chunk i uses IDX[(sc, i)]
IDX = {}
for _s in range(NT):
    for _i in range(_s + R + 1, NT):
        IDX[(_s, _i)] = len(IDX)
NTR = len(IDX)    # 15 for R=2
F32 = mybir.dt.float32
BF16 = mybir.dt.bfloat16
EXP = mybir.ActivationFunctionType.Exp

_CACHED_NC = None


def build_nc():
    nc = bacc.Bacc("TRN2", target_bir_lowering=False)

    # Inputs are host-repacked so every SBUF partition's data is one
    # contiguous DRAM run (big DMA bursts instead of 2KB packets):
    # xr[p, k*S+s] = x[s, 128k+p], w0/w1 likewise for W^T columns 0:128
    # (the pair-0 block, loaded first) and 128:768.
    xr = nc.dram_tensor("xr", [128, NP * S], BF16, kind="ExternalInput")
    w0 = nc.dram_tensor("w0", [128, NP * 128], BF16, kind="ExternalInput")
    w1 = nc.dram_tensor("w1", [128, NP * 640], BF16, kind="ExternalInput")
    bias_d = nc.dram_tensor("bias_d", [128, NP], F32, kind="ExternalInput")
    mask_d = nc.dram_tensor("mask_d", [128, NT], F32, kind="ExternalInput")
    ident_d = nc.dram_tensor("ident_d", [128, 128], BF16, kind="ExternalInput")
    out_d = nc.dram_tensor("out", [S, D], F32, kind="ExternalOutput")

    with tile.TileContext(nc) as tc:
        with (
            tc.tile_pool(name="consts", bufs=1) as consts,
            tc.tile_pool(name="big", bufs=1) as big,
            tc.tile_pool(name="upool", bufs=18) as upool,
            tc.tile_pool(name="rpool", bufs=8) as rpool,
            tc.tile_pool(name="utpool", bufs=2) as utpool,
            tc.tile_pool(name="ps_big", bufs=2, space="PSUM") as ps_big,
            tc.tile_pool(name="ps_ctx", bufs=1, space="PSUM") as ps_ctx,
            tc.tile_pool(name="ps_sm", bufs=2, space="PSUM") as ps_sm,
        ):
            identbf = consts.tile([128, 128], BF16)
            nc.scalar.dma_start(out=identbf, in_=ident_d[:, :])
            bias_t = consts.tile([128, NP], F32)
            nc.gpsimd.dma_start(out=bias_t, in_=bias_d[:, :])
            mask_t = consts.tile([128, NT], F32)
            nc.gpsimd.dma_start(out=mask_t, in_=mask_d[:, :])

            # Phase-1 DMA: three big partition-contiguous transfers (the
            # j=0 weight block and all of x first; remaining weight columns
            # stream in behind on the third queue).
            wts0 = big.tile([128, NP, 128], BF16)
            wts1 = big.tile([128, NP, 640], BF16)
            xts = big.tile([128, NP, S], BF16)
            nc.sync.dma_start(out=wts0[:, :, :], in_=w0[:, :])
            nc.sync.dma_start(out=xts[:, 0:2, :], in_=xr[:, 0:2 * S])
            nc.scalar.dma_start(out=xts[:, 2:4, :], in_=xr[:, 2 * S:4 * S])
            nc.gpsimd.dma_start(out=xts[:, 4:NP, :], in_=xr[:, 4 * S:NP * S])
            nc.gpsimd.dma_start(out=wts1[:, :, :], in_=w1[:, :])

            # Keep the PE clock ramping while inputs stream in.
            for w in range(10):
                ptw = ps_sm.tile([128, 128], BF16, name="pt", bufs=1)
                nc.tensor.transpose(ptw, identbf, identbf)

            # emask[t] = exp(mask[t]); folded into xl rows (exact mask).
            emask = consts.tile([128, NT], F32)
            nc.scalar.activation(out=emask, in_=mask_t, func=EXP)

            mixbf = big.tile([128, NP, S], BF16)
            stages = big.tile([128, NT, H, DH], F32)

            # xl[t, i, q, 0:64] = emask[t] * v values; col 64 = emask[t]
            # (the ones column pre-scaled by the mask factor). Ping-pong per
            # head pair: the deferred ctx of pair j reads its slot while
            # prep(j+1) writes the other.
            xlts = [big.tile([128, NT, 2, DH + 1], BF16, name=f"xl{p}")
                    for p in range(2)]
            for xlt in xlts:
                # cols 0:64 are fully overwritten by the prep evacuations;
                # only the masked ones column needs initialization.
                nc.vector.tensor_copy(
                    out=xlt[:, :, :, DH],
                    in_=emask[:, :, None].broadcast_to((128, NT, 2)),
                )

            # Persistent ping-pong Z tiles; zero halves are set once.
            zt = [[big.tile([128, S], BF16, name=f"z{q}{p}") for p in range(2)]
                  for q in range(2)]
            for q in range(2):
                olo = (1 - q) * 64
                for p in range(2):
                    nc.gpsimd.memset(zt[q][p][olo:olo + 64, :], 0.0)

            # Preload the ACT exp table while the inputs stream in.
            warm = consts.tile([128, 8], F32)
            nc.scalar.activation(out=warm, in_=mask_t[:, 0:8], func=EXP,
                                 scale=0.125)

            def prep(j):
                """Projection + Z/xl staging for head pair j."""
                pp = j % 2
                # prep(0) is the ramp critical path: the ctx psum tile is
                # idle before the first head, so run both projection halves
                # concurrently in its two banks.
                pm0 = ps_ctx.tile([128, 2, 4, 128], F32, name="pc") \
                    if j == 0 else None
                for n in range(2):
                    if j == 0:
                        pm = pm0[:, n, :, :]
                    else:
                        pm = ps_sm.tile([128, 512], F32, name="pm", bufs=1)
                    for k in range(NP):
                        wsl = wts0[:, k, :] if j == 0 else \
                            wts1[:, k, (j - 1) * 128:j * 128]
                        nc.tensor.matmul(
                            pm,
                            lhsT=wsl,
                            rhs=xts[:, k, n * 512:(n + 1) * 512],
                            start=(k == 0),
                            stop=(k == NP - 1),
                        )
                    nc.vector.tensor_scalar_add(
                        mixbf[:, j, n * 512:(n + 1) * 512], pm,
                        bias_t[:, j:j + 1]
                    )
                    # z copies split per half so the first scores chunk can
                    # start as soon as the first evacuation lands.
                    for q in range(2):
                        lo = q * 64
                        nc.vector.tensor_copy(
                            out=zt[q][pp][lo:lo + 64,
                                          n * 512:(n + 1) * 512],
                            in_=mixbf[lo:lo + 64, j,
                                      n * 512:(n + 1) * 512],
                        )
                zs = [zt[0][pp], zt[1][pp]]
                xlt = xlts[pp]
                for i in range(NT):
                    pt = ps_sm.tile([128, 128], BF16, name="pt", bufs=1)
                    nc.tensor.transpose(
                        pt, mixbf[:, j, i * 128:(i + 1) * 128], identbf
                    )
                    # [128, 2, 64] masked-scaled evacuation into xl
                    nc.vector.tensor_scalar_mul(
                        xlt[:, i, :, 0:DH],
                        pt[:, :].rearrange("p (q d) -> p q d", q=2),
                        emask[:, i:i + 1],
                    )
                return zs

            def scores_chunk(j, q, zs, i, ut):
                """Banded scores + exp for t-chunk i of head (j, q): only
                columns >= C0[i]; the skipped tiles of other chunks are
                produced here by transposing this chunk's columns into ut.
                Returns the u tile [128, S] (region [C0[i]:] valid)."""
                c0 = C0[i]
                psc = ps_big.tile([128, S], F32, name="psc")
                for n in range(2):
                    lo = max(c0, n * 512)
                    hi = (n + 1) * 512
                    if lo >= hi:
                        continue
                    nc.tensor.matmul(
                        psc[:, lo:hi],
                        lhsT=zs[q][:, i * 128:(i + 1) * 128],
                        rhs=mixbf[:, j, lo:hi],
                        start=True,
                        stop=True,
                    )
                u = upool.tile([128, S], BF16, name="u")
                nc.scalar.activation(
                    out=u[:, c0:], in_=psc[:, c0:], func=EXP, scale=0.125,
                )
                # U^T tiles for the chunks whose exp skips column-block i
                dsts = [c for c in range(i + R + 1, NT)]
                if dsts:
                    pt = ps_sm.tile([128, NT, 128], BF16, name="pt", bufs=1)
                    for m, c in enumerate(dsts):
                        nc.tensor.transpose(
                            pt[:, m, :], u[:, c * 128:(c + 1) * 128], identbf)
                    m0 = IDX[(i, dsts[0])]
                    nc.vector.tensor_copy(
                        out=ut[:, m0:m0 + len(dsts), :],
                        in_=pt[:, 0:len(dsts), :])
                return u

            def ctx_head(h, us, ut, pc, half=None):
                """ctx for head h: stationary-U matmuls accumulating each
                s-chunk psum slot [128, DH+1] over the 8 t-chunks. Slot
                groups run sequentially (one open psum group per bank).
                half=0/1 emits only slots 0-3 / 4-7 (so the PE flood can be
                split around the next head's scores)."""
                xlt = xlts[(h // 2) % 2]
                q = h % 2
                scs = range(NT) if half is None else \
                    range(half * 4, half * 4 + 4)
                for sc in scs:
                    for i in range(NT):
                        if i - sc > R:
                            lhsT = ut[:, IDX[(sc, i)], :]
                        else:
                            lhsT = us[i][:, sc * 128:(sc + 1) * 128]
                        nc.tensor.matmul(
                            pc[:, sc // 4, sc % 4, 0:DH + 1],
                            lhsT=lhsT,
                            rhs=xlt[:, i, q, :],
                            start=(i == 0),
                            stop=(i == NT - 1),
                        )

            def epilogue(h, pc):
                """Normalize the 8 s-chunk slots of head h into stages."""
                den = rpool.tile([128, 2, 4], F32, name="den")
                nc.vector.tensor_copy(out=den, in_=pc[:, :, :, DH])
                rcp = rpool.tile([128, 2, 4], F32, name="rcp")
                nc.vector.reciprocal(out=rcp, in_=den)
                for sc in range(NT):
                    nc.vector.tensor_scalar_mul(
                        stages[:, sc, h, :],
                        pc[:, sc // 4, sc % 4, 0:DH],
                        rcp[:, sc // 4, sc % 4:sc % 4 + 1],
                    )

            def epilogue_half(h, pc, b, fengs):
                """Normalize+flush one bank-half of the last head while the
                other half's ctx matmuls still run."""
                den = rpool.tile([128, 4], F32, name="denh")
                nc.vector.tensor_copy(out=den, in_=pc[:, b, :, DH])
                rcp = rpool.tile([128, 4], F32, name="rcph")
                nc.vector.reciprocal(out=rcp, in_=den)
                for k in range(4):
                    sc = b * 4 + k
                    nc.vector.tensor_scalar_mul(
                        stages[:, sc, h, :],
                        pc[:, b, k, 0:DH],
                        rcp[:, k:k + 1],
                    )
                    fengs[sc % 3].dma_start(
                        out=out_d[sc * 128:(sc + 1) * 128, h * 64:(h + 1) * 64],
                        in_=stages[:, sc, h:h + 1, :],
                    )

            def flush(h0, h1, final=False):
                engs = [nc.sync, nc.gpsimd, nc.scalar] if final else \
                    [nc.sync, nc.gpsimd]
                for sj in range(NT):
                    engs[sj % len(engs)].dma_start(
                        out=out_d[sj * 128:(sj + 1) * 128, h0 * 64:h1 * 64],
                        in_=stages[:, sj, h0:h1, :],
                    )

            zs = prep(0)
            pending = None  # (h, q, us) awaiting ctx + epilogue
            for j in range(NP):
                for q in range(2):
                    h = 2 * j + q
                    last = (h == 2 * NP - 1)
                    if last:
                        # Final head: emit its scores/exps, drain the pending
                        # head's ctx while they run, then this head's ctx +
                        # epilogue trail the last ACTIVATE.
                        ut = utpool.tile([128, NTR, 128], BF16, name="ut")
                        us = [scores_chunk(j, q, zs, i, ut) for i in range(NT)]
                        ph, pq, pus, put = pending
                        pc_prev = ps_ctx.tile([128, 2, 4, 128], F32, name="pc")
                        ctx_head(ph, pus, put, pc_prev)
                        epilogue(ph, pc_prev)
                        flush(6, 11)
                        pc = ps_ctx.tile([128, 2, 4, 128], F32, name="pc")
                        fengs = [nc.sync, nc.gpsimd, nc.scalar]
                        ctx_head(h, us, ut, pc, half=0)
                        epilogue_half(h, pc, 0, fengs)
                        ctx_head(h, us, ut, pc, half=1)
                        epilogue_half(h, pc, 1, fengs)
                        continue
                    ut = utpool.tile([128, NTR, 128], BF16, name="ut")
                    us = [scores_chunk(j, q, zs, i, ut) for i in range(6)]
                    if q == 1 and j + 1 < NP:
                        # Emit the next pair's prep mid-head: the PE gets the
                        # projection work while ACT still has queued exps,
                        # and the next head's scores are ready the moment
                        # this head's exps drain.
                        zs_next = prep(j + 1)
                    else:
                        zs_next = zs
                    for i in range(6, NT):
                        us.append(scores_chunk(j, q, zs, i, ut))
                    if pending is not None:
                        ph, pq, pus, put = pending
                        pc = ps_ctx.tile([128, 2, 4, 128], F32, name="pc")
                        ctx_head(ph, pus, put, pc)
                        epilogue(ph, pc)
                        if ph == 5:
                            flush(0, 6)
                    pending = (h, q, us, ut)
                    zs = zs_next

    nc.compile()
    return nc


def kernel(x, attention_mask, W, b, _profile=None):
    global _CACHED_NC
    if _CACHED_NC is None:
        _CACHED_NC = build_nc()
    nc = _CACHED_NC

    x = np.asarray(x, dtype=np.float32)
    attention_mask = np.asarray(attention_mask, dtype=np.float32)
    W = np.asarray(W, dtype=np.float32)
    b = np.asarray(b, dtype=np.float32)

    import ml_dtypes

    # Partition-contiguous repacks (see build_nc): [p, k*cols+c] = T[128k+p, c]
    wT = W.T.astype(ml_dtypes.bfloat16).reshape(NP, 128, D)
    w0 = np.ascontiguousarray(
        wT[:, :, 0:128].transpose(1, 0, 2).reshape(128, NP * 128))
    w1 = np.ascontiguousarray(
        wT[:, :, 128:D].transpose(1, 0, 2).reshape(128, NP * 640))
    bias_cols = np.ascontiguousarray(b.reshape(NP, 128).T)
    ident = np.eye(128, dtype=ml_dtypes.bfloat16)

    in_maps = []
    for i in range(B):
        xr = np.ascontiguousarray(
            x[i].T.astype(ml_dtypes.bfloat16).reshape(NP, 128, S)
            .transpose(1, 0, 2).reshape(128, NP * S))
        in_maps.append({
            "xr": xr,
            "w0": w0,
            "w1": w1,
            "bias_d": bias_cols,
            "mask_d": np.ascontiguousarray(
                attention_mask[i, 0, 0].reshape(NT, 128).T
            ),
            "ident_d": ident,
        })

    kwargs = dict(_profile) if _profile else {}
    res = run_bass_kernel_spmd(nc, in_maps, core_ids=list(range(B)), **kwargs)
    out = np.stack([res.results[i]["out"] for i in range(B)], axis=0)
    if _profile:
        kernel.last_results = res
    return out


if __name__ == "__main__":
    rng = np.random.default_rng(0)
    x = rng.standard_normal((B, S, D), dtype=np.float32)
    m = np.zeros((B, 1, 1, S), dtype=np.float32)
    W = (rng.standard_normal((D, D), dtype=np.float32) / np.sqrt(D)).astype(np.float32)
    b = np.zeros((D,), dtype=np.float32)
    out = kernel(x, m, W, b)
    print("out", out.shape, out.dtype)

